# revision 23
# baseline (speedup 1.0000x reference)
"""Trainium2 Bass kernel for nn_EConly_85469849190489 (dual-branch molecular
transformer). Data-parallel over batch: 8 NeuronCores x 4 molecules each.

Layout strategy (per core):
  - residual stream x kept tokens-major fp32 (cheap LayerNorm via bn_stats)
  - feature-major fp16 copies (xT) made via PE transpose for matmul moving use
  - attention computed transposed: scores_T(k,q) with k on partitions, so the
    pad mask is a per-partition ACT bias, exp needs no max pass, and A_raw^T
    feeds the A@V matmul with no transposes of A
  - softmax denominators via ones-matmuls on PE (col-packed into 32-row strips
    of one PSUM bank) + reciprocal_approx_fast on DVE
  - attention probability tensors (P^T, exp(dist)^T, adj^T, A^T) in fp16 for
    DVE 2x mode; all matmul weights fp16; fp32 operands bitcast to float32r
"""

import numpy as np
from contextlib import ExitStack

import concourse.bass as bass
import concourse.tile as tile
from concourse import mybir
from concourse.tile import TileContext, ScopedClock

F16 = mybir.dt.float16
F32 = mybir.dt.float32
F32R = mybir.dt.float32r
AF = mybir.ActivationFunctionType
ALU = mybir.AluOpType

B, S_FULL, F_IN, D, H, L, DFF, C = 32, 512, 64, 256, 8, 4, 1024, 4
U1, U2 = 512, 256
NH, DEPTH, DH = 4, 32, 128
NEG = -1.0e9
EPS_LN = 1.0e-6
NCORES = 1
MPC = 32  # molecules per core; streamed one at a time, weights SBUF-resident
P = 128

_PATCHED = False


def _patch_drain():
    """This walrus build allows only one sync-wait per CTRL instruction; the
    TileContext exit drain carries one wait per live semaphore. Spread the
    extras across single-wait SP nops."""
    global _PATCHED
    if _PATCHED:
        return

    def _drain_and_barrier(self, tick_clock, wait_clock):
        nc = self.nc
        drain_inst = nc.sync.drain()
        wait_clock.add_sem_waits(
            drain_inst.ins, ScopedClock({None: tick_clock.global_clock})
        )
        si = drain_inst.ins.sync_info
        waits = list(si.on_wait) if si is not None else []
        if len(waits) > 1:
            si.on_wait = waits[:1]
            drain_inst.ins.sync_info = si
            for w in waits[1:]:
                nop = nc.sync.nop(nofuse=True)
                nop.ins.sync_info = mybir.SyncInfo(on_wait=[w], on_update=[])
        nc.all_engine_barrier()
        popped = nc._tile_sem_poison_stack.pop()
        assert popped is self._sem_poison
        nc.clear_and_free_semaphores(list(self.sems.allocated().values()))
        nc.all_engine_barrier()

    TileContext._drain_and_barrier = _drain_and_barrier
    _PATCHED = True


def _split_multiwaits(nc):
    """This walrus build allows only one sync-wait per instruction: move extra
    waits onto same-engine nops placed immediately before the instruction."""
    n = 0
    for fn in nc.m.functions:
        for blk in fn.blocks:
            newl = []
            for inst in blk.instructions:
                si = inst.sync_info
                if si is not None and len(si.on_wait) > 1:
                    waits = list(si.on_wait)
                    for i, w in enumerate(waits[:-1]):
                        nop = mybir.InstNoOp(
                            name=f"{inst.name}-w{i}", ins=[], outs=[], engine=inst.engine
                        )
                        nop.sync_info = mybir.SyncInfo(on_wait=[w], on_update=[])
                        newl.append(nop)
                        n += 1
                    si.on_wait = waits[-1:]
                    inst.sync_info = si
                newl.append(inst)
            blk.instructions = newl
    return n


def _chunks(S):
    """[(c, p0, pc)] partition chunks covering S: offset p0, size pc<=128."""
    out = []
    c = 0
    while c * P < S:
        out.append((c, c * P, min(P, S - c * P)))
        c += 1
    return out


def _blob_layout(slot_S):
    """All per-molecule data rides in ONE u8 input tensor per core (fewer
    runtime tensor bindings per exec). Returns ({section: offset}, total)."""
    off = 0
    lay = {}

    def add(key, nbytes):
        nonlocal off
        lay[key] = off
        off += (nbytes + 511) // 512 * 512

    for m, S in enumerate(slot_S):
        KC = len(_chunks(S))
        add(f"adjT{m}", S * S)
        add(f"distT{m}", S * S)
        add(f"mft{m}", 64 * S * 2)
        add(f"negc{m}", KC * 128 * 4)
        add(f"poolm{m}", KC * 128 * 4)
    return lay, off


def build_program(slot_S, weights, L_run=L, taps=(), time_loop=0, extra_inputs=0):
    """Build the SPMD per-core program. slot_S: list of MPC sequence lengths
    (each a multiple of 8, <= 512). weights: dict of prepped numpy arrays
    (from _prep_weights) baked into the NEFF as Const tensors so they are
    staged to HBM once at model load instead of re-uploaded per execution.
    taps: debug tensor names to stream out."""
    _patch_drain()
    nc = bass.Bass(trn_type="TRN2", detect_race_conditions=False)

    dram = {}
    taps = set(taps)

    def tap(name, ap, shape=None):
        if name not in taps:
            return
        shape = list(shape if shape is not None else ap.shape)
        t = nc.dram_tensor("dbg_" + name, shape, ap.dtype, kind="ExternalOutput")
        nc.sync.dma_start(out=t[...], in_=ap)

    def din(name, shape, dt):
        dram[name] = nc.dram_tensor(name, list(shape), dt, kind="ExternalInput")
        return dram[name]

    U8 = mybir.dt.uint8
    lay, blob_total = _blob_layout(slot_S)
    blob = din("blob", (blob_total,), U8)

    def u8_ap(key, r0, rc, rowb):
        """(rc, rowb) u8 rows r0..r0+rc of a rowb-byte-strided section."""
        o = lay[key] + r0 * rowb
        return blob[o : o + rc * rowb].rearrange("(p s) -> p s", s=rowb)

    def f16_ap(key, rows, cols):
        o = lay[key]
        return (
            blob[o : o + rows * cols * 2].bitcast(F16).rearrange("(p s) -> p s", s=cols)
        )

    def f32col_ap(key, KC):
        """(128, KC, 1) f32 view of (KC,128,1)-stored section."""
        o = lay[key]
        return (
            blob[o : o + KC * 128 * 4]
            .bitcast(F32)
            .rearrange("(c p) -> p c", p=128)
            .unsqueeze(-1)
        )

    for k in (
        "wembed",
        "wq",
        "wk",
        "wv",
        "wo",
        "wf1",
        "wf2",
        "wh1",
        "wh2",
        "ident32",
        "ident16",
    ):
        dram[k] = nc.inline_tensor(weights[k], k)
    din("chain", (C, MPC, U2), F32)
    for i in range(extra_inputs):
        din(f"dummy{i}", (512,), U8)
    out_dram = nc.dram_tensor("out", [C, MPC, U2], F32, kind="ExternalOutput")

    with TileContext(nc) as tc:
        with ExitStack() as ctx:
            pers = ctx.enter_context(tc.tile_pool(name="pers", bufs=1))
            mol = ctx.enter_context(tc.tile_pool(name="mol", bufs=2))
            wk2 = ctx.enter_context(tc.tile_pool(name="work", bufs=2))
            wk3 = ctx.enter_context(tc.tile_pool(name="work3", bufs=3))
            ps_s = ctx.enter_context(tc.tile_pool(name="ps_s", bufs=2, space="PSUM"))
            ps_r = ctx.enter_context(tc.tile_pool(name="ps_r", bufs=1, space="PSUM"))
            ps_o = ctx.enter_context(tc.tile_pool(name="ps_o", bufs=1, space="PSUM"))
            ps_g = ctx.enter_context(tc.tile_pool(name="ps_g", bufs=2, space="PSUM"))

            # ---- persistent constants / weights (all layers SBUF-resident) ----
            ident_32 = pers.tile([128, 128], F32, tag="id32")
            nc.sync.dma_start(out=ident_32, in_=dram["ident32"][:, :])
            ident_16 = pers.tile([128, 128], F16, tag="id16")
            nc.sync.dma_start(out=ident_16, in_=dram["ident16"][:, :])
            ones16 = pers.tile([128, 128], F16, tag="ones16")
            nc.vector.memset(ones16, 1.0)
            eps_col = pers.tile([128, 1], F32, tag="eps")
            nc.vector.memset(eps_col, EPS_LN)
            chain_sb = pers.tile([128, C * MPC * U2 // 128], F32, tag="chain")
            nc.sync.dma_start(
                out=chain_sb, in_=dram["chain"].rearrange("k m u -> (k m) u")
            )
            if extra_inputs:
                dscr = pers.tile([1, 512], mybir.dt.uint8, tag="dscr")
                for i in range(extra_inputs):
                    nc.sync.dma_start(
                        out=dscr[0:1, :], in_=dram[f"dummy{i}"][...].unsqueeze(0)
                    )

            wq_l, wk_l, wv_l, wo_l, wf1_l, wf2_l = [], [], [], [], [], []
            for l in range(L):
                t = pers.tile([128, 2, 2, DH], F16, tag=f"wq{l}")
                nc.sync.dma_start(
                    out=t, in_=dram["wq"][l].rearrange("b (c p) n -> p b c n", p=128)
                )
                wq_l.append(t)
                t = pers.tile([128, 2, 2, DH], F16, tag=f"wk{l}")
                nc.sync.dma_start(
                    out=t, in_=dram["wk"][l].rearrange("b (c p) n -> p b c n", p=128)
                )
                wk_l.append(t)
                t = pers.tile([128, 2, 2 * DH], F16, tag=f"wv{l}")
                nc.sync.dma_start(
                    out=t, in_=dram["wv"][l].rearrange("(c p) n -> p c n", p=128)
                )
                wv_l.append(t)
                t = pers.tile([128, 2, D], F16, tag=f"wo{l}")
                nc.sync.dma_start(
                    out=t, in_=dram["wo"][l].rearrange("(c p) n -> p c n", p=128)
                )
                wo_l.append(t)
                t = pers.tile([128, 2, DFF], F16, tag=f"wf1{l}")
                nc.sync.dma_start(
                    out=t, in_=dram["wf1"][l].rearrange("(c p) n -> p c n", p=128)
                )
                wf1_l.append(t)
                t = pers.tile([128, 8, D], F16, tag=f"wf2{l}")
                nc.sync.dma_start(
                    out=t, in_=dram["wf2"][l].rearrange("(c p) n -> p c n", p=128)
                )
                wf2_l.append(t)

            wh1_sb = pers.tile([128, C, 2, U1], F16, tag="wh1")
            nc.sync.dma_start(
                out=wh1_sb, in_=dram["wh1"].rearrange("k (c p) n -> p k c n", p=128)
            )
            wh2_sb = pers.tile([128, C, 4, U2], F16, tag="wh2")
            nc.sync.dma_start(
                out=wh2_sb, in_=dram["wh2"].rearrange("k (c p) n -> p k c n", p=128)
            )
            wembed_sb = pers.tile([128, D], F16, tag="wembed")
            nc.sync.dma_start(out=wembed_sb[0:64, :], in_=dram["wembed"][:, :])

            # pooled^T columns accumulate here, one per molecule
            pT_sb = pers.tile([128, 2, MPC], F16, tag="pT")

            MM = nc.tensor.matmul

            from contextlib import nullcontext
            with (tc.For_i(0, time_loop, 1) if time_loop else nullcontext()):
                # ---- molecule stream: phase0 + L layers + pooling, one mol at a time ----
                for m, S in enumerate(slot_S):
                    ch = _chunks(S)
                    KC = len(ch)

                    negc = mol.tile([128, KC, 1], F32, tag="negc", name=f"negc{m}")
                    nc.sync.dma_start(out=negc, in_=f32col_ap(f"negc{m}", KC))
                    poolm = mol.tile([128, KC, 1], F32, tag="poolm", name=f"poolm{m}")
                    nc.sync.dma_start(out=poolm, in_=f32col_ap(f"poolm{m}", KC))
                    mft = mol.tile([128, S], F16, tag="mft", name=f"mft{m}")
                    nc.sync.dma_start(out=mft[0:64, :], in_=f16_ap(f"mft{m}", 64, S))
                    adjT = mol.tile([128, KC, S], F16, tag="adjT", name=f"adjT{m}")
                    for c, p0, pc in ch:
                        u = wk2.tile([128, 512], mybir.dt.uint8, tag="u8stage")
                        nc.sync.dma_start(
                            out=u[0:pc, 0:S], in_=u8_ap(f"adjT{m}", p0, pc, S)
                        )
                        nc.vector.tensor_copy(out=adjT[0:pc, c, :], in_=u[0:pc, 0:S])
                    EdT = mol.tile([128, KC, S], F16, tag="EdT", name=f"EdT{m}")
                    rdblk = mol.tile([128, S], F32, tag="rdblk", name=f"rdblk{m}")
                    xT = mol.tile([128, 2, S], F16, tag="xT", name=f"xT{m}")
                    xtok = mol.tile([128, KC, D], F32, tag="xtok", name=f"xtok{m}")

                    # phase 0: embed + exp(dist^T + neg) + dist row-sum block
                    for c in range(2):
                        pse = ps_g.tile([128, 512], F32, tag="gp")
                        MM(
                            pse[:, 0:S],
                            wembed_sb[0:64, c * 128 : (c + 1) * 128],
                            mft[0:64, :],
                            start=True,
                            stop=True,
                        )
                        nc.vector.tensor_copy(out=xT[:, c, :], in_=pse[:, 0:S])
                    for c, p0, pc in ch:
                        pse = ps_g.tile([128, 512], F32, tag="gp")
                        MM(
                            pse[0:pc, 0:D],
                            mft[0:64, p0 : p0 + pc],
                            wembed_sb[0:64, :],
                            start=True,
                            stop=True,
                        )
                        nc.vector.tensor_copy(out=xtok[0:pc, c, :], in_=pse[0:pc, 0:D])
                    # E_d^T = exp(dist^T + neg) fp16 (dist staged as uint8,
                    # dequantized by the ACT scale), and its row-sum block
                    for c, p0, pc in ch:
                        du = wk2.tile([128, 512], mybir.dt.uint8, tag="distu8")
                        nc.sync.dma_start(
                            out=du[0:pc, 0:S], in_=u8_ap(f"distT{m}", p0, pc, S)
                        )
                        dsb = wk2.tile([128, S], F16, tag="dist", name="dsb")
                        nc.vector.tensor_copy(out=dsb[0:pc, :], in_=du[0:pc, 0:S])
                        nc.scalar.activation(
                            EdT[0:pc, c, :],
                            dsb[0:pc, :],
                            AF.Exp,
                            bias=negc[0:pc, c, :],
                            scale=1.0 / 255.0,
                        )
                    prd = ps_r.tile([128, 512], F32, tag="r")
                    for c, p0, pc in ch:
                        MM(
                            prd[:, 0:S],
                            ones16[0:pc, :],
                            EdT[0:pc, c, :],
                            start=(c == 0),
                            stop=(c == KC - 1),
                            skip_group_check=True,
                        )
                    nc.vector.tensor_copy(out=rdblk[:, :], in_=prd[:, 0:S])

                    if m == 0:
                        tap("xT0", xT[:, :, :])
                        tap("xtok0", xtok[:, :, :])
                        tap("EdT0", EdT[:, :, :])
                        tap("rdblk0", rdblk[:, :])

                    # ---- layers ----
                    for l in range(L_run):
                        wq_sb, wk_sb, wv_sb = wq_l[l], wk_l[l], wv_l[l]
                        wo_sb, wf1_sb, wf2_sb = wo_l[l], wf1_l[l], wf2_l[l]

                        # q^T, k^T projections (feature-major fp16), per branch
                        qT, kT = [], []
                        for br in range(2):
                            for which, dst_list, w_sb in (
                                (0, qT, wq_sb),
                                (1, kT, wk_sb),
                            ):
                                pp = ps_g.tile([128, 512], F32, tag="gp")
                                for c in range(2):
                                    MM(
                                        pp[:, 0:S],
                                        w_sb[:, br, c, :],
                                        xT[:, c, :],
                                        start=(c == 0),
                                        stop=(c == 1),
                                    )
                                t = wk2.tile([128, S], F16, tag=("qT" if which == 0 else "kT") + str(br), name="qkT")
                                nc.vector.tensor_copy(out=t[:, :], in_=pp[:, 0:S])
                                if l == 0 and m == 0:
                                    tap(f"{'qT' if which == 0 else 'kT'}{br}", t[:, :])
                                dst_list.append(t)

                        # V tokens-major fp16 (both branches concatenated)
                        Vt = wk2.tile([128, KC, 2 * DH], F16, tag="Vt")
                        for c, p0, pc in ch:
                            pv = ps_g.tile([128, 512], F32, tag="gp")
                            for cc in range(2):
                                MM(
                                    pv[0:pc, 0 : 2 * DH],
                                    xT[:, cc, p0 : p0 + pc],
                                    wv_sb[:, cc, :],
                                    start=(cc == 0),
                                    stop=(cc == 1),
                                )
                            nc.vector.tensor_copy(out=Vt[0:pc, c, :], in_=pv[0:pc, 0 : 2 * DH])
                        if l == 0 and m == 0:
                            tap("Vt0", Vt[:, :, :])

                        # attention per branch: 0=dist, 1=adj
                        osc = wk2.tile([128, 2, S], F16, tag="osc")
                        for br in range(2):
                            po = ps_o.tile([128, 512], F32, tag="o")
                            pr = ps_r.tile([128, 512], F32, tag="r")
                            # per-strip start=True initializes each 32-row strip
                            first_o = [True] * 4
                            first_r = [True] * 4
                            for c, p0, pc in ch:
                                for pair in range(2):
                                    ps = ps_s.tile([128, 2, 512], F32, tag="s")
                                    for i in range(2):
                                        h = 2 * pair + i
                                        MM(
                                            ps[0:pc, i, 0:S],
                                            kT[br][32 * h : 32 * h + 32, p0 : p0 + pc],
                                            qT[br][32 * h : 32 * h + 32, :],
                                            start=True,
                                            stop=True,
                                            tile_position=(32 * h, 0),
                                        )
                                    PT = wk3.tile([128, 2, S], F16, tag="PT")
                                    nc.scalar.activation(
                                        PT[0:pc, :, :],
                                        ps[0:pc, :, 0:S],
                                        AF.Exp,
                                        bias=negc[0:pc, c, :],
                                        scale=1.0,
                                    )
                                    if l == 0 and m == 0 and c == 0 and pair == 0:
                                        tap(f"PT0_br{br}", PT[:, :, :])
                                    for i in range(2):
                                        h = 2 * pair + i
                                        MM(
                                            pr[32 * h : 32 * h + 32, 0:S],
                                            ones16[0:pc, 0:32],
                                            PT[0:pc, i, :],
                                            start=first_r[h],
                                            stop=(c == KC - 1 and pair == 1 and i == 1),
                                            tile_position=(0, 32 * h),
                                            skip_group_check=True,
                                        )
                                        first_r[h] = False
                                    MT = EdT if br == 0 else adjT
                                    Aw = wk3.tile([128, 2, S], F16, tag="Aw")
                                    for i in range(2):
                                        nc.vector.tensor_mul(
                                            Aw[0:pc, i, :],
                                            PT[0:pc, i, :],
                                            MT[0:pc, c, :],
                                        )
                                    for i in range(2):
                                        h = 2 * pair + i
                                        MM(
                                            po[32 * h : 32 * h + 32, 0:S],
                                            Vt[0:pc, c, br * DH + 32 * h : br * DH + 32 * h + 32],
                                            Aw[0:pc, i, :],
                                            start=first_o[h],
                                            stop=(c == KC - 1 and pair == 1 and i == 1),
                                            tile_position=(0, 32 * h),
                                            skip_group_check=True,
                                        )
                                        first_o[h] = False
                            # row scaling: osc = po / denom
                            rrt = wk2.tile([128, S], F32, tag="rrt")
                            if br == 0:
                                nc.vector.tensor_mul(rrt[:, :], pr[:, 0:S], rdblk[:, :])
                            else:
                                nc.vector.tensor_copy(out=rrt[:, :], in_=pr[:, 0:S])
                            if l == 0 and m == 0:
                                tap(f"rrt{br}", rrt[:, :])
                            recipb = wk2.tile([128, S], F32, tag="recipb")
                            nc.vector.reciprocal(out=recipb[:, :], in_=rrt[:, :])
                            nc.vector.tensor_mul(osc[:, br, :], po[:, 0:S], recipb[:, :])
                        if l == 0 and m == 0:
                            tap("osc0", osc[:, :, :])

                        # W_o (tokens-major) + residual + LN1
                        xres = wk2.tile([128, KC, D], F32, tag="xres")
                        xln1 = wk2.tile([128, KC, D], F32, tag="xln1")
                        mv = wk2.tile([128, KC, 2], F32, tag="mv")
                        if S % 128:
                            nc.vector.memset(mv, 0.0)
                        for c, p0, pc in ch:
                            pw = ps_g.tile([128, 512], F32, tag="gp")
                            for cc in range(2):
                                MM(
                                    pw[0:pc, 0:D],
                                    osc[:, cc, p0 : p0 + pc],
                                    wo_sb[:, cc, :],
                                    start=(cc == 0),
                                    stop=(cc == 1),
                                )
                            nc.vector.tensor_add(
                                xres[0:pc, c, :], pw[0:pc, 0:D], xtok[0:pc, c, :]
                            )
                            bst = wk3.tile([128, 6], F32, tag="bst")
                            nc.vector.bn_stats(out=bst[0:pc, :], in_=xres[0:pc, c, :])
                            nc.vector.bn_aggr(out=mv[0:pc, c, :], in_=bst[0:pc, :])
                        if l == 0 and m == 0:
                            tap("xres0", xres[:, :, :])
                        sd = wk2.tile([128, KC], F32, tag="sd")
                        nc.scalar.activation(
                            sd[:, :], mv[:, :, 1], AF.Sqrt, bias=eps_col[:, :], scale=1.0
                        )
                        rstd = wk2.tile([128, KC], F32, tag="rstd")
                        nc.vector.reciprocal(out=rstd[:, :], in_=sd[:, :])
                        for c, p0, pc in ch:
                            nc.vector.tensor_scalar(
                                out=xln1[0:pc, c, :],
                                in0=xres[0:pc, c, :],
                                scalar1=mv[0:pc, c, 0:1],
                                scalar2=rstd[0:pc, c : c + 1],
                                op0=ALU.subtract,
                                op1=ALU.mult,
                            )
                        if l == 0 and m == 0:
                            tap("xln1_0", xln1[:, :, :])
                        # x_ln1^T (feature-major fp16) via PE transpose
                        xln1T = wk2.tile([128, 2, S], F16, tag="xln1T")
                        for cc in range(2):
                            pt = ps_g.tile([128, 512], F32, tag="gp")
                            for c, p0, pc in ch:
                                nc.tensor.transpose(
                                    pt[:, p0 : p0 + pc],
                                    xln1[0:pc, c, cc * 128 : (cc + 1) * 128],
                                    ident_32[0:pc, 0:pc],
                                )
                            nc.vector.tensor_copy(out=xln1T[:, cc, :], in_=pt[:, 0:S])

                        if l == 0 and m == 0:
                            tap("xln1T0", xln1T[:, :, :])
                        # FFN1 (feature-major) with relu -> h^T fp16
                        hT = wk2.tile([128, 8, S], F16, tag="hT")
                        for hc in range(8):
                            pf = ps_g.tile([128, 512], F32, tag="gp")
                            for cc in range(2):
                                MM(
                                    pf[:, 0:S],
                                    wf1_sb[:, cc, hc * 128 : (hc + 1) * 128],
                                    xln1T[:, cc, :],
                                    start=(cc == 0),
                                    stop=(cc == 1),
                                )
                            nc.vector.tensor_scalar(
                                out=hT[:, hc, :],
                                in0=pf[:, 0:S],
                                scalar1=0.0,
                                scalar2=None,
                                op0=ALU.max,
                            )
                        if l == 0 and m == 0:
                            tap("hT0", hT[:, :, :])
                        # FFN2 (feature-major) -> transpose -> +residual -> LN2
                        ff2T = wk2.tile([128, 2, S], F16, tag="ff2T")
                        for oc in range(2):
                            pf = ps_g.tile([128, 512], F32, tag="gp")
                            for hc in range(8):
                                MM(
                                    pf[:, 0:S],
                                    wf2_sb[:, hc, oc * 128 : (oc + 1) * 128],
                                    hT[:, hc, :],
                                    start=(hc == 0),
                                    stop=(hc == 7),
                                )
                            nc.vector.tensor_copy(out=ff2T[:, oc, :], in_=pf[:, 0:S])
                        mv2 = wk2.tile([128, KC, 2], F32, tag="mv2")
                        if S % 128:
                            nc.vector.memset(mv2, 0.0)
                        xres2 = wk2.tile([128, KC, D], F32, tag="xres2")
                        for c, p0, pc in ch:
                            pt = ps_g.tile([128, 512], F16, tag="gp")
                            for cc in range(2):
                                nc.tensor.transpose(
                                    pt[0:pc, cc * 128 : (cc + 1) * 128],
                                    ff2T[:, cc, p0 : p0 + pc],
                                    ident_16[:, :],
                                )
                            nc.vector.tensor_add(
                                xres2[0:pc, c, :], pt[0:pc, 0:D], xln1[0:pc, c, :]
                            )
                            bst = wk3.tile([128, 6], F32, tag="bst")
                            nc.vector.bn_stats(out=bst[0:pc, :], in_=xres2[0:pc, c, :])
                            nc.vector.bn_aggr(out=mv2[0:pc, c, :], in_=bst[0:pc, :])
                        sd2 = wk2.tile([128, KC], F32, tag="sd2")
                        nc.scalar.activation(
                            sd2[:, :], mv2[:, :, 1], AF.Sqrt, bias=eps_col[:, :], scale=1.0
                        )
                        rstd2 = wk2.tile([128, KC], F32, tag="rstd2")
                        nc.vector.reciprocal(out=rstd2[:, :], in_=sd2[:, :])
                        for c, p0, pc in ch:
                            nc.vector.tensor_scalar(
                                out=xtok[0:pc, c, :],
                                in0=xres2[0:pc, c, :],
                                scalar1=mv2[0:pc, c, 0:1],
                                scalar2=rstd2[0:pc, c : c + 1],
                                op0=ALU.subtract,
                                op1=ALU.mult,
                            )
                        if l == 0 and m == 0:
                            tap("xtok_l0", xtok[:, :, :])
                        if l < L_run - 1:
                            # x^T for next layer
                            for cc in range(2):
                                pt = ps_g.tile([128, 512], F32, tag="gp")
                                for c, p0, pc in ch:
                                    nc.tensor.transpose(
                                        pt[:, p0 : p0 + pc],
                                        xtok[0:pc, c, cc * 128 : (cc + 1) * 128],
                                        ident_32[0:pc, 0:pc],
                                    )
                                nc.vector.tensor_copy(out=xT[:, cc, :], in_=pt[:, 0:S])

                    # pooled^T column for this molecule: xtok^T @ poolmask
                    ppool = ps_g.tile([128, 512], F32, tag="gp")
                    for cc in range(2):
                        for ci, (c, p0, pc) in enumerate(ch):
                            MM(
                                ppool[:, cc : cc + 1],
                                xtok[0:pc, c, cc * 128 : (cc + 1) * 128],
                                poolm[0:pc, c, :],
                                start=(ci == 0),
                                stop=(ci == KC - 1),
                                skip_group_check=True,
                            )
                    nc.vector.tensor_copy(out=pT_sb[:, :, m], in_=ppool[:, 0:2])

                # ---- contrastive heads over all molecules ----
                for k in range(C):
                    h1 = wk2.tile([128, 4, MPC], F16, tag="h1")
                    for u in range(4):
                        ph = ps_g.tile([128, 512], F32, tag="gp")
                        for cc in range(2):
                            MM(
                                ph[:, 0:MPC],
                                wh1_sb[:, k, cc, u * 128 : (u + 1) * 128],
                                pT_sb[:, cc, :],
                                start=(cc == 0),
                                stop=(cc == 1),
                            )
                        nc.vector.tensor_scalar(
                            out=h1[:, u, :],
                            in0=ph[:, 0:MPC],
                            scalar1=0.0,
                            scalar2=None,
                            op0=ALU.max,
                        )
                    h2m = wk2.tile([128, D], F32, tag="h2m")
                    ph2 = ps_g.tile([128, 512], F32, tag="gp")
                    for oc in range(2):
                        for uc in range(4):
                            MM(
                                ph2[:, oc * MPC : (oc + 1) * MPC],
                                wh2_sb[:, k, uc, oc * 128 : (oc + 1) * 128],
                                h1[:, uc, :],
                                start=(uc == 0),
                                stop=(uc == 3),
                            )
                    # relu into fp16 tile, transpose to (MPC x 256) rows
                    h2f = wk2.tile([128, 2, MPC], F16, tag="h2f")
                    for oc in range(2):
                        nc.vector.tensor_scalar(
                            out=h2f[:, oc, :],
                            in0=ph2[:, oc * MPC : (oc + 1) * MPC],
                            scalar1=0.0,
                            scalar2=None,
                            op0=ALU.max,
                        )
                    pht = ps_g.tile([128, 512], F16, tag="gp")
                    for oc in range(2):
                        nc.tensor.transpose(
                            pht[0:MPC, oc * 128 : (oc + 1) * 128],
                            h2f[:, oc, :],
                            ident_16[:, :],
                        )
                    nc.vector.tensor_copy(out=h2m[0:MPC, :], in_=pht[0:MPC, 0:D])
                    # l2 normalize rows
                    sq = wk2.tile([128, D], F32, tag="sq")
                    nc.vector.tensor_mul(sq[0:MPC, :], h2m[0:MPC, :], h2m[0:MPC, :])
                    ss = wk2.tile([128, 1], F32, tag="ss")
                    nc.vector.reduce_sum(ss[0:MPC, :], sq[0:MPC, :], axis=mybir.AxisListType.X)
                    nc.vector.tensor_scalar(
                        out=ss[0:MPC, :],
                        in0=ss[0:MPC, :],
                        scalar1=1.0e-12,
                        scalar2=None,
                        op0=ALU.max,
                    )
                    srt = wk2.tile([128, 1], F32, tag="srt")
                    nc.scalar.activation(srt[0:MPC, :], ss[0:MPC, :], AF.Sqrt)
                    rs = wk2.tile([128, 1], F32, tag="rs")
                    nc.vector.reciprocal(out=rs[0:MPC, :], in_=srt[0:MPC, :])
                    fin = wk2.tile([128, D], F32, tag="fin")
                    nc.vector.tensor_scalar(
                        out=fin[0:MPC, :],
                        in0=h2m[0:MPC, :],
                        scalar1=rs[0:MPC, :],
                        scalar2=None,
                        op0=ALU.mult,
                    )
                    nc.sync.dma_start(out=out_dram[k], in_=fin[0:MPC, :])

    _split_multiwaits(nc)
    return nc


# ----------------------------------------------------------------------------
# host side
# ----------------------------------------------------------------------------


def _prep_weights(inputs):
    f16 = np.float16
    wq = np.stack(
        [inputs["W_qkv"][:, 0] / np.sqrt(DEPTH), inputs["W_qkv"][:, 3] / np.sqrt(DEPTH)],
        axis=1,
    ).astype(f16)
    wk = np.stack([inputs["W_qkv"][:, 1], inputs["W_qkv"][:, 4]], axis=1).astype(f16)
    wv = np.concatenate([inputs["W_qkv"][:, 2], inputs["W_qkv"][:, 5]], axis=-1).astype(
        f16
    )
    return {
        "wembed": inputs["W_embed"].astype(f16),
        "wq": wq,
        "wk": wk,
        "wv": wv,
        "wo": inputs["W_o"].astype(f16),
        "wf1": inputs["W_ff1"].astype(f16),
        "wf2": inputs["W_ff2"].astype(f16),
        "wh1": inputs["Wh1"].astype(f16),
        "wh2": inputs["Wh2"].astype(f16),
        "ident32": np.eye(128, dtype=np.float32),
        "ident16": np.eye(128, dtype=np.float16),
    }


def _check_trivial(inputs):
    z = [
        "b_embed",
        "b_qkv",
        "b_o",
        "b_ff1",
        "b_ff2",
        "bh1",
        "bh2",
        "ln1_b",
        "ln2_b",
    ]
    ok = all(np.abs(inputs[k]).max() == 0.0 for k in z)
    ok = ok and np.all(inputs["ln1_g"] == 1.0) and np.all(inputs["ln2_g"] == 1.0)
    if not ok:
        raise NotImplementedError(
            "kernel specialized for zero biases / unit layernorm gains (per spec)"
        )


def _mol_arrays(b_idx, inputs, S, perm=None):
    """Per-molecule prepped arrays, optionally token-permuted, truncated to S."""
    mol = np.asarray(inputs["mol_feat"][b_idx])
    adj = np.asarray(inputs["adj"][b_idx])
    dist = np.asarray(inputs["dist"][b_idx])
    mask = np.asarray(inputs["mask"][b_idx, 0, 0, :])
    if perm is not None:
        mol, adj, dist, mask = (
            mol[perm],
            adj[perm][:, perm],
            dist[perm][:, perm],
            mask[perm],
        )
    mol, adj, dist, mask = mol[:S], adj[:S, :S], dist[:S, :S], mask[:S]
    KC = len(_chunks(S))
    negc = np.full((KC, 128, 1), NEG, np.float32)
    poolm = np.zeros((KC, 128, 1), np.float32)
    negflat = (mask * NEG).astype(np.float32)
    poolflat = (mask == 0).astype(np.float32)
    for c, p0, pc in _chunks(S):
        negc[c, 0:pc, 0] = negflat[p0 : p0 + pc]
        poolm[c, 0:pc, 0] = poolflat[p0 : p0 + pc]
    return {
        "mft": np.ascontiguousarray(mol.T).astype(np.float16),
        "adjT": np.ascontiguousarray(adj.T).astype(np.uint8),
        "distT": np.ascontiguousarray(np.round(dist.T * 255.0)).astype(np.uint8),
        "negc": negc,
        "poolm": poolm,
    }


def plan(inputs):
    """Sort molecules by real-token count; slot s of every core gets one of the
    8 molecules ranked [8s, 8s+8); slot length = roundup8(max real in group)."""
    mask = np.asarray(inputs["mask"])[:, 0, 0, :]
    real = (mask == 0).sum(1)
    order = np.argsort(-real, kind="stable")
    slot_S, assign = [], [[0] * MPC for _ in range(NCORES)]
    for s in range(MPC):
        group = order[NCORES * s : NCORES * (s + 1)]
        Smax = int(min(((int(real[group].max()) + 7) // 8) * 8, S_FULL))
        slot_S.append(Smax)
        for c in range(NCORES):
            assign[c][s] = int(group[c])
    return slot_S, assign


def make_in_maps(inputs, slot_S, assign, compact=True):
    """assign[c][m] = molecule index for core c, slot m. Per-molecule data is
    packed into one u8 blob per core (plus the chain tensor); weights ride
    inside the NEFF as constants."""
    mask_all = np.asarray(inputs["mask"])[:, 0, 0, :]
    lay, blob_total = _blob_layout(slot_S)
    in_maps = []
    for c in range(NCORES):
        blob = np.zeros((blob_total,), np.uint8)
        for m in range(MPC):
            b = assign[c][m]
            perm = None
            if compact:
                perm = np.argsort(mask_all[b], kind="stable")
            arrs = _mol_arrays(b, inputs, slot_S[m], perm=perm)
            for k, v in arrs.items():
                raw = np.ascontiguousarray(v).view(np.uint8).reshape(-1)
                o = lay[f"{k}{m}"]
                blob[o : o + raw.size] = raw
        in_maps.append(
            {"blob": blob, "chain": np.zeros((C, MPC, U2), np.float32)}
        )
    return in_maps


def kernel(**inputs):
    _check_trivial(inputs)
    slot_S, assign = plan(inputs)
    nc = build_program(slot_S, _prep_weights(inputs))
    in_maps = make_in_maps(inputs, slot_S, assign)
    from concourse.bass_utils import run_bass_kernel_spmd

    res = run_bass_kernel_spmd(nc, in_maps, core_ids=list(range(NCORES)))
    out = np.zeros((C, B, U2), np.float32)
    for c in range(NCORES):
        o = res.results[c]["out"]  # (C, MPC, U2)
        for m in range(MPC):
            out[:, assign[c][m], :] = o[:, m, :]
    return out

